# revision 1
# baseline (speedup 1.0000x reference)
"""BitNet FFN (bitlinear158 -> gelu -> bitlinear158) Trainium2 kernel.

Sharding: data-parallel over tokens across 8 cores (1024 tokens/core).
Layout: tokens on the free axis everywhere; weights stationary in the PE.

Math notes (exactness):
  - activation quant ints = round(x * 127 / max|x|)  (the rms-norm cancels)
  - weight quant ternary = clip(round(w / clip(mean|w|,1e-5)), -1, 1)
  - both exactly representable in bf16; PSUM accumulates integer products
    (<= 2^21) exactly in fp32, so the matmuls are exact.
  - per-token output scale alpha = clip(max|x|*sqrt(d)/||x||, 1e-5)
      * clip(mean|w|,1e-5) / 127 applied on PSUM before gelu.
  - round-to-nearest-even via fp32 (t + 1.5*2^23) - 1.5*2^23, matching
    jnp.round; clip(round(t),-1,1) == round(clamp(t, +-1.4999999)).
  - mean|w| needs the full tensor: each core reduces its row-shard, then a
    tiny AllReduce combines the partial sums.
"""

import sys

for _p in ("/opt/trn_rl_repo", "/opt/trn_rl_repo/concourse"):
    if _p not in sys.path:
        sys.path.insert(0, _p)

import numpy as np

import concourse.bass as bass
import concourse.bacc as bacc
import concourse.mybir as mybir
import concourse.tile as tile
from concourse import library_config
from concourse.bass import ts
from concourse.masks import make_identity

F32 = mybir.dt.float32
BF16 = mybir.dt.bfloat16
AX = mybir.AxisListType.X
OP = mybir.AluOpType
AF = mybir.ActivationFunctionType

C_ROUND = 12582912.0  # 1.5 * 2**23 : fp32 RNE rounding constant
W_CLIP = 1.4999999    # round(clamp(t, +-W_CLIP)) == clip(round(t), -1, 1)
N_CORES = 8


def build_bitnet(D, I, T, n_cores=N_CORES, gelu_mode="gelu"):
    """Per-core SPMD Bass program.

    Per-core I/O: xT [D,T] f32 (token shard, transposed), w1T [D,I] f32 and
    w2T [I,D] f32 (full transposed weights), w1s [D/n,I] / w2s [I/n,D]
    (this core's rows, for the mean|w| partial) -> outT [D,T] f32.
    """
    KD = D // 128   # d tiles (layer-1 contraction; layer-2 output rows)
    KI = I // 128   # inner tiles
    TH = T // 2     # matmul moving free dim
    TJ = T // 128   # token tiles for stats transposes
    JD2 = D // 256  # paired output-column strips in layer 2
    K2H = KI // 2   # half of inner tiles (layer-2 weight streaming)
    R1 = D // n_cores   # w1 shard rows per core
    R2 = I // n_cores   # w2 shard rows per core
    A1 = (R1 + 127) // 128
    A2 = (R2 + 127) // 128
    inv_cnt = 1.0 / float(D * I)
    sqrt_d = float(np.sqrt(np.float64(D)))
    sqrt_i = float(np.sqrt(np.float64(I)))

    nc = bacc.Bacc("TRN2", num_devices=n_cores)

    xT = nc.dram_tensor("xT", [D, T], F32, kind="ExternalInput")
    w1T = nc.dram_tensor("w1T", [D, I], F32, kind="ExternalInput")
    w2T = nc.dram_tensor("w2T", [I, D], F32, kind="ExternalInput")
    w1s = nc.dram_tensor("w1s", [D // n_cores, I], F32, kind="ExternalInput")
    w2s = nc.dram_tensor("w2s", [I // n_cores, D], F32, kind="ExternalInput")
    outT = nc.dram_tensor("outT", [D, T], F32, kind="ExternalOutput")

    h_dram = nc.dram_tensor("h_scratch", [I, T], F32, kind="Internal")
    w1ag_in = nc.dram_tensor("w1ag_in", [R1, I], BF16, kind="Internal")
    w1q_dram = nc.dram_tensor("w1q_ag", [D, I], BF16, kind="Internal",
                              addr_space="Shared")
    w2ag_in = nc.dram_tensor("w2ag_in", [R2, D], BF16, kind="Internal")
    w2q_dram = nc.dram_tensor("w2q_ag", [I, D], BF16, kind="Internal",
                              addr_space="Shared")
    ar_in = nc.dram_tensor("ar_in", [8], F32, kind="Internal")
    ar_out = nc.dram_tensor("ar_out", [8], F32, kind="Internal",
                            addr_space="Shared")
    stat_dram = nc.dram_tensor("stat_dram", [6, T], F32, kind="Internal")
    srow_v = stat_dram.ap()                                     # [6, T]
    stok_v = stat_dram.ap().rearrange("r (j p) -> r p j", p=128)  # [6,128,TJ]

    xT_t = xT.ap().rearrange("(k p) t -> k p t", p=128)           # [KD,128,T]
    w1_t = w1T.ap().rearrange("(k p) (i j) -> i p k j", p=128, j=128)
    w1s_ap = w1s.ap()
    w2s_ap = w2s.ap()
    w1q_t = w1q_dram.ap().rearrange("(k p) (i j) -> i p k j", p=128, j=128)
    w2q_r = w2q_dram.ap().rearrange("(k p) (m c) -> m p k c", p=128, c=256)
    h_w = h_dram.ap().rearrange("(k p) t -> k p t", p=128)
    out_w = outT.ap().rearrange("(k p) t -> k p t", p=128)

    with tile.TileContext(nc) as tc:
        with (
            tc.tile_pool(name="glob", bufs=1) as glob,
            tc.tile_pool(name="psum", bufs=8, space="PSUM") as psum,
            tc.tile_pool(name="stats", bufs=1) as stats,
        ):
            # --- persistent constants & small tiles ---
            ident = glob.tile([128, 128], F32)
            make_identity(nc, ident)
            wsc = glob.tile([128, 4], F32)   # cols: s1, s2, mclip1, mclip2
            qs1_b = glob.tile([128, T], F32, tag="qsb")
            al1_b = glob.tile([128, T], F32, tag="alb")

            # stats layout shuffles go through DRAM rows: token t = 128*j + p

            def part_reduce(acc, res, op):
                # reduce [128, T] over partitions -> res [128, TJ] tok-part
                for j in range(TJ):
                    trp = psum.tile([128, 128], F32, tag="b", name="trp")
                    nc.tensor.transpose(trp[:, :], acc[:, ts(j, 128)],
                                        ident[:, :])
                    nc.vector.tensor_reduce(
                        out=res[:, j:j + 1], in_=trp[:, :], axis=AX, op=op)

            def finalize_stats(Mx, ssq, mclip_col, sqrt_dim, qs_b, al_b, r0):
                """Mx/ssq [128,TJ] tok-part absmax / sumsq.
                Builds qs_b = 127/max|x| and al_b = per-token dequant scale,
                both broadcast to [128, T]. r0: base row in stat_dram."""
                nrm = stats.tile([128, TJ], F32, name="nrm")
                nc.vector.tensor_scalar(nrm, ssq, 1e-38, None, OP.max)
                nc.scalar.activation(nrm, nrm, AF.Sqrt)
                nc.vector.tensor_scalar(nrm, nrm, 1e-12, None, OP.max)
                inv_n = stats.tile([128, TJ], F32, name="inv_n")
                nc.vector.reciprocal(inv_n, nrm)
                al = stats.tile([128, TJ], F32, name="al")
                nc.vector.tensor_tensor(al, Mx, inv_n, OP.mult)
                nc.vector.tensor_scalar(al, al, sqrt_dim, 1e-5, OP.mult, OP.max)
                nc.vector.tensor_scalar(al, al, wsc[:, mclip_col:mclip_col + 1],
                                        1.0 / 127.0, OP.mult, OP.mult)
                qs = stats.tile([128, TJ], F32, name="qs")
                nc.vector.tensor_scalar(qs, Mx, 1e-30, None, OP.max)
                nc.vector.reciprocal(qs, qs)
                nc.vector.tensor_scalar(qs, qs, 127.0, None, OP.mult)
                nc.sync.dma_start(out=stok_v[r0 + 1], in_=qs[:, :])
                nc.sync.dma_start(out=stok_v[r0 + 2], in_=al[:, :])
                qrow = stats.tile([1, T], F32, name="qrow")
                arow = stats.tile([1, T], F32, name="arow")
                nc.sync.dma_start(out=qrow[:, :], in_=srow_v[r0 + 1:r0 + 2, :])
                nc.sync.dma_start(out=arow[:, :], in_=srow_v[r0 + 2:r0 + 3, :])
                nc.gpsimd.partition_broadcast(qs_b[:, :], qrow[:, :])
                nc.gpsimd.partition_broadcast(al_b[:, :], arow[:, :])

            # ========= Stage A: weight scale partials + AllReduce =========
            with tc.tile_pool(name="wredp", bufs=2) as wredp:
                wps = stats.tile([128, A1 + A2], F32)
                if R1 % 128 or R2 % 128:  # partial chunks need zero padding
                    nc.vector.memset(wps, 0.0)
                for a in range(A1):
                    pp = min(128, R1 - 128 * a)
                    wtmp = wredp.tile([128, I], F32, tag="wred", name="wtmp")
                    nc.sync.dma_start(out=wtmp[:pp, :],
                                      in_=w1s_ap[128 * a:128 * a + pp, :])
                    nc.vector.tensor_reduce(
                        out=wps[:pp, a:a + 1], in_=wtmp[:pp, :], axis=AX,
                        op=OP.add, apply_absolute_value=True)
                for a in range(A2):
                    pp = min(128, R2 - 128 * a)
                    wtmp2 = wredp.tile([128, I], F32, tag="wred", name="wtmp2")
                    nc.sync.dma_start(out=wtmp2[:pp, :D],
                                      in_=w2s_ap[128 * a:128 * a + pp, :])
                    nc.vector.tensor_reduce(
                        out=wps[:pp, A1 + a:A1 + a + 1], in_=wtmp2[:pp, :D],
                        axis=AX, op=OP.add, apply_absolute_value=True)
                wpad = stats.tile([128, 128], F32)
                nc.vector.memset(wpad, 0.0)
                nc.vector.reduce_sum(wpad[:, 0:1], wps[:, 0:A1], axis=AX)
                nc.vector.reduce_sum(wpad[:, 1:2], wps[:, A1:A1 + A2], axis=AX)
                trw = psum.tile([128, 128], F32, tag="b", name="trw")
                nc.tensor.transpose(trw[:, :], wpad[:, :], ident[:, :])
                wred = stats.tile([8, 1], F32)
                nc.vector.memset(wred, 0.0)
                nc.vector.reduce_sum(wred[0:2, :], trw[0:2, :], axis=AX)
                nc.sync.dma_start(out=ar_in.ap()[0:8], in_=wred[:, :])
                nc.gpsimd.collective_compute(
                    "AllReduce", OP.add,
                    replica_groups=[list(range(n_cores))],
                    ins=[ar_in.ap().opt()], outs=[ar_out.ap().opt()])
                wrow = stats.tile([1, 2], F32)
                nc.sync.dma_start(out=wrow[:, :], in_=ar_out.ap()[0:2])
                mrow = stats.tile([1, 4], F32)
                nc.vector.tensor_scalar(mrow[:, 2:4], wrow[:, :], inv_cnt,
                                        1e-5, OP.mult, OP.max)
                nc.vector.reciprocal(mrow[:, 0:2], mrow[:, 2:4])
                nc.gpsimd.partition_broadcast(wsc[:, :], mrow[:, :])

                # quantize this core's shards (ternary bf16), then AllGather
                def quant_shard(src_ap, rows, width, scol, dst_ap):
                    for a in range((rows + 127) // 128):
                        pp = min(128, rows - 128 * a)
                        wqf = wredp.tile([128, I], F32, tag="wred",
                                         name="wqf")
                        nc.sync.dma_start(
                            out=wqf[:pp, :width],
                            in_=src_ap[128 * a:128 * a + pp, :])
                        nc.scalar.activation(wqf[:pp, :width],
                                             wqf[:pp, :width], AF.Copy,
                                             scale=wsc[:pp, scol:scol + 1])
                        nc.vector.tensor_scalar(wqf[:pp, :width],
                                                wqf[:pp, :width], W_CLIP,
                                                -W_CLIP, OP.min, OP.max)
                        wqb = wredp.tile([128, I], BF16, tag="wqb",
                                         name="wqb")
                        nc.vector.tensor_scalar(wqb[:pp, :width],
                                                wqf[:pp, :width], C_ROUND,
                                                C_ROUND, OP.add, OP.subtract)
                        nc.sync.dma_start(
                            out=dst_ap[128 * a:128 * a + pp, :],
                            in_=wqb[:pp, :width])

                quant_shard(w1s_ap, R1, I, 0, w1ag_in.ap())
                nc.gpsimd.collective_compute(
                    "AllGather", OP.bypass,
                    replica_groups=[list(range(n_cores))],
                    ins=[w1ag_in.ap().opt()], outs=[w1q_dram.ap().opt()])
                quant_shard(w2s_ap, R2, D, 1, w2ag_in.ap())
                nc.gpsimd.collective_compute(
                    "AllGather", OP.bypass,
                    replica_groups=[list(range(n_cores))],
                    ins=[w2ag_in.ap().opt()], outs=[w2q_dram.ap().opt()])

            with tc.tile_pool(name="bc", bufs=2) as bc:
                # ================= Stage B: x stats + quant =================
                am1p = stats.tile([128, T], F32, tag="amp", name="am1p")
                am1n = stats.tile([128, T], F32, tag="amn", name="am1n")
                sq1 = stats.tile([128, T], F32, tag="sq", name="sq1")
                for k in range(KD):
                    xk = bc.tile([128, T], F32, tag="xk", name="xk")
                    nc.sync.dma_start(out=xk[:, :], in_=xT_t[k])
                    if k == 0:
                        nc.vector.tensor_copy(am1p, xk)
                        nc.vector.tensor_copy(am1n, xk)
                    else:
                        nc.vector.tensor_tensor(am1p, xk, am1p, OP.max)
                        nc.vector.tensor_tensor(am1n, xk, am1n, OP.min)
                    xsq = bc.tile([128, T], BF16, tag="xsq", name="xsq")
                    nc.scalar.activation(xsq, xk, AF.Square)
                    if k == 0:
                        nc.vector.tensor_copy(sq1, xsq)
                    else:
                        nc.vector.tensor_tensor(sq1, xsq, sq1, OP.add)
                nc.vector.scalar_tensor_tensor(
                    am1n, am1n, -1.0, am1p, OP.mult, OP.max)
                Mx1 = stats.tile([128, TJ], F32)
                part_reduce(am1n, Mx1, OP.max)
                Sq1 = stats.tile([128, TJ], F32)
                part_reduce(sq1, Sq1, OP.add)
                finalize_stats(Mx1, Sq1, 2, sqrt_d, qs1_b, al1_b, 0)

                xqT = bc.tile([128, KD, T], BF16, tag="xqT", bufs=1,
                              name="xqT")
                for k in range(KD):
                    xk2 = bc.tile([128, T], F32, tag="xk", name="xk2")
                    nc.sync.dma_start(out=xk2[:, :], in_=xT_t[k])
                    nc.vector.tensor_tensor(xk2, xk2, qs1_b, OP.mult)
                    nc.vector.tensor_scalar(xqT[:, k, :], xk2, C_ROUND,
                                            C_ROUND, OP.add, OP.subtract)

                # ===== Stage C: layer 1 + h stats + w2 quant (interleaved) ====
                am2p = stats.tile([128, T], F32, tag="amp", name="am2p")
                am2n = stats.tile([128, T], F32, tag="amn", name="am2n")
                sq2 = stats.tile([128, T], F32, tag="sq2", name="sq2")
                n_jit = min(16, KI)  # strips quantized locally while the
                for i in range(KI):  # w1q AllGather is still in flight
                    w1q = bc.tile([128, KD, 128], BF16, tag="w1q", name="w1q")
                    if i < n_jit:
                        w1f = bc.tile([128, KD, 128], F32, tag="w1f",
                                      name="w1f")
                        nc.sync.dma_start(out=w1f[:, :, :], in_=w1_t[i])
                        w1ff = w1f.rearrange("p k j -> p (k j)")
                        nc.scalar.activation(w1ff, w1ff, AF.Copy,
                                             scale=wsc[:, 0:1])
                        nc.vector.tensor_scalar(w1ff, w1ff, W_CLIP, -W_CLIP,
                                                OP.min, OP.max)
                        nc.vector.tensor_scalar(
                            w1q.rearrange("p k j -> p (k j)"), w1ff, C_ROUND,
                            C_ROUND, OP.add, OP.subtract)
                    else:
                        nc.sync.dma_start(out=w1q[:, :, :], in_=w1q_t[i])
                    hpsA = psum.tile([128, TH], F32, tag="b", name="hpsA")
                    hpsB = psum.tile([128, TH], F32, tag="b", name="hpsB")
                    for k in range(KD):
                        nc.tensor.matmul(hpsA[:, :], w1q[:, k, :],
                                         xqT[:, k, 0:TH],
                                         start=(k == 0), stop=(k == KD - 1))
                    for k in range(KD):
                        nc.tensor.matmul(hpsB[:, :], w1q[:, k, :],
                                         xqT[:, k, TH:T],
                                         start=(k == 0), stop=(k == KD - 1))
                    nc.vector.tensor_tensor(hpsA, hpsA, al1_b[:, 0:TH], OP.mult)
                    nc.vector.tensor_tensor(hpsB, hpsB, al1_b[:, TH:T], OP.mult)
                    h_sb = bc.tile([128, T], F32, tag="h", bufs=3, name="h_sb")
                    if gelu_mode == "gelu":
                        nc.scalar.activation(h_sb[:, 0:TH], hpsA, AF.Gelu)
                        nc.scalar.activation(h_sb[:, TH:T], hpsB, AF.Gelu)
                    else:  # sigmoid-gelu (CoreSim lacks Gelu/Erf tables)
                        gs = bc.tile([128, T], F32, tag="gsig", name="gs")
                        nc.scalar.activation(gs[:, 0:TH], hpsA, AF.Sigmoid,
                                             scale=1.702)
                        nc.scalar.activation(gs[:, TH:T], hpsB, AF.Sigmoid,
                                             scale=1.702)
                        nc.vector.tensor_tensor(h_sb[:, 0:TH], gs[:, 0:TH],
                                                hpsA, OP.mult)
                        nc.vector.tensor_tensor(h_sb[:, TH:T], gs[:, TH:T],
                                                hpsB, OP.mult)
                    nc.sync.dma_start(out=h_w[i], in_=h_sb[:, :])
                    if i == 0:
                        nc.vector.tensor_copy(am2p, h_sb)
                        nc.vector.tensor_copy(am2n, h_sb)
                    else:
                        nc.vector.tensor_tensor(am2p, h_sb, am2p, OP.max)
                        nc.vector.tensor_tensor(am2n, h_sb, am2n, OP.min)
                    hsq = bc.tile([128, T], BF16, tag="hsq", name="hsq")
                    nc.scalar.activation(hsq, h_sb, AF.Square)
                    if i == 0:
                        nc.vector.tensor_copy(sq2, hsq)
                    else:
                        nc.vector.tensor_tensor(sq2, hsq, sq2, OP.add)

                # ---- mid stats finalize ----
                qs2_b = glob.tile([128, T], F32, tag="qsb", name="qs2_b")
                al2_b = glob.tile([128, T], F32, tag="alb", name="al2_b")
                nc.vector.scalar_tensor_tensor(
                    am2n, am2n, -1.0, am2p, OP.mult, OP.max)
                Mx2 = stats.tile([128, TJ], F32, name="Mx2")
                part_reduce(am2n, Mx2, OP.max)
                Sq2 = stats.tile([128, TJ], F32, name="Sq2")
                part_reduce(sq2, Sq2, OP.add)
                finalize_stats(Mx2, Sq2, 3, sqrt_i, qs2_b, al2_b, 3)

            # ================= Stage D: quantize h, layer 2 =================
            with tc.tile_pool(name="l2", bufs=2) as l2:
                hqT = l2.tile([128, KI, T], BF16, tag="hqT", bufs=1,
                              name="hqT")
                for k2 in range(KI):
                    hk = l2.tile([128, T], F32, tag="hrd", name="hk")
                    nc.sync.dma_start(out=hk[:, :], in_=h_w[k2])
                    nc.vector.tensor_tensor(hk, hk, qs2_b, OP.mult)
                    nc.vector.tensor_scalar(hqT[:, k2, :], hk, C_ROUND,
                                            C_ROUND, OP.add, OP.subtract)
                for m in range(JD2):
                    pbank = [psum.tile([128, TH], F32, tag="b",
                                       name=f"psb{q}") for q in range(4)]
                    for kh in range(2):
                        w2qs = l2.tile([128, K2H, 256], BF16, tag="w2s",
                                       bufs=2, name="w2qs")
                        nc.sync.dma_start(
                            out=w2qs[:, :, :],
                            in_=w2q_r[m][:, ts(kh, K2H), :])
                        for kk in range(K2H):
                            k2 = kh * K2H + kk
                            first = (k2 == 0)
                            last = (k2 == KI - 1)
                            nc.tensor.matmul(pbank[0][:, :], w2qs[:, kk, 0:128],
                                             hqT[:, k2, 0:TH],
                                             start=first, stop=last)
                            nc.tensor.matmul(pbank[1][:, :], w2qs[:, kk, 0:128],
                                             hqT[:, k2, TH:T],
                                             start=first, stop=last)
                            nc.tensor.matmul(pbank[2][:, :],
                                             w2qs[:, kk, 128:256],
                                             hqT[:, k2, 0:TH],
                                             start=first, stop=last)
                            nc.tensor.matmul(pbank[3][:, :],
                                             w2qs[:, kk, 128:256],
                                             hqT[:, k2, TH:T],
                                             start=first, stop=last)
                    for jcol in range(2):
                        for half in range(2):
                            ob = l2.tile([128, TH], F32, tag="ob", bufs=2,
                                         name="ob")
                            nc.vector.tensor_tensor(
                                ob, pbank[2 * jcol + half],
                                al2_b[:, ts(half, TH)], OP.mult)
                            nc.sync.dma_start(
                                out=out_w[2 * m + jcol][:, ts(half, TH)],
                                in_=ob[:, :])

    nc.compile()  # Bacc passes: EVSEM multi-wait lowering, library loads,
    return nc     # extended-ISA codegen, nop fusion, register alloc


_NC_CACHE = {}


def _get_nc(D, I, T, n_cores):
    key = (D, I, T, n_cores)
    if key not in _NC_CACHE:
        _NC_CACHE[key] = build_bitnet(D, I, T, n_cores)
    return _NC_CACHE[key]


def make_in_maps(x, w1, w2, n_cores=N_CORES):
    """Host-side sharding/layout only (transpose + slicing, no arithmetic)."""
    xf = np.ascontiguousarray(np.asarray(x, dtype=np.float32)).reshape(
        -1, x.shape[-1])
    D = xf.shape[1]
    I = w1.shape[0]
    T = xf.shape[0] // n_cores
    w1T = np.ascontiguousarray(np.asarray(w1, dtype=np.float32).T)  # [D, I]
    w2T = np.ascontiguousarray(np.asarray(w2, dtype=np.float32).T)  # [I, D]
    in_maps = []
    for c in range(n_cores):
        xTc = np.ascontiguousarray(xf[c * T:(c + 1) * T].T)  # [D, T]
        in_maps.append({
            "xT": xTc,
            "w1T": w1T,
            "w2T": w2T,
            "w1s": np.ascontiguousarray(
                w1T[c * (D // n_cores):(c + 1) * (D // n_cores)]),
            "w2s": np.ascontiguousarray(
                w2T[c * (I // n_cores):(c + 1) * (I // n_cores)]),
        })
    return in_maps, (D, I, T)


def run_spmd(x, w1, w2, trace=False, **kwargs):
    from concourse.bass_utils import run_bass_kernel_spmd

    B, S, D = x.shape
    in_maps, (D, I, T) = make_in_maps(x, w1, w2, N_CORES)
    nc = _get_nc(D, I, T, N_CORES)
    res = run_bass_kernel_spmd(nc, in_maps, core_ids=list(range(N_CORES)),
                               trace=trace, **kwargs)
    outs = [res.results[c]["outT"].T for c in range(N_CORES)]  # each [T, D]
    out = np.concatenate(outs, axis=0).reshape(B, S, D)
    return np.ascontiguousarray(out, dtype=np.float32), res


def kernel(x, w1, w2):
    out, _ = run_spmd(x, w1, w2, trace=False)
    return out



# revision 3
# speedup vs baseline: 1.1018x; 1.1018x over previous
"""BitNet FFN (bitlinear158 -> gelu -> bitlinear158) Trainium2 kernel, v2.

Sharding: data-parallel over tokens across 8 cores (1024 tokens/core).
Layout: tokens on the free axis everywhere; weights stationary in the PE.

v2 structure (vs v1): the w1 AllGather that serialized ~450us of prologue is
gone -- each core JIT-quantizes w1 bands locally from its own full f32 copy
(only the tiny 8-float weight-scale AllReduce remains on the critical path).
w2 is still shard-quantized + AllGathered, but that now overlaps under the
~440us of layer-1 matmuls.  Both layers issue matmuls k-innermost in small
PSUM ping-pong groups so the PE never waits on PSUM drain, and consecutive
matmuls share the stationary operand (2 MMs per LDWEIGHTS).

Math notes (exactness, same as v1):
  - activation quant ints = round(x * 127 / max|x|)  (the rms-norm cancels)
  - weight quant ternary = clip(round(w / clip(mean|w|,1e-5)), -1, 1)
  - both exactly representable in bf16; PSUM accumulates integer products
    exactly in fp32, so the matmuls are exact.
  - per-token output scale alpha = clip(max|x|*sqrt(d)/||x||, 1e-5)
      * clip(mean|w|,1e-5) / 127 applied on PSUM before gelu.
  - round-to-nearest-even via fp32 (t + 1.5*2^23) - 1.5*2^23, matching
    jnp.round; round-then-clip == reference clip(round(t), -1, 1).
  - mean|w| needs the full tensor: each core reduces its row-shard, then a
    tiny AllReduce combines the partial sums.
"""

import sys

for _p in ("/opt/trn_rl_repo", "/opt/trn_rl_repo/concourse"):
    if _p not in sys.path:
        sys.path.insert(0, _p)

import numpy as np

import concourse.bass as bass
import concourse.bacc as bacc
import concourse.mybir as mybir
import concourse.tile as tile
from concourse.bass import ts
from concourse.masks import make_identity

F32 = mybir.dt.float32
BF16 = mybir.dt.bfloat16
AX = mybir.AxisListType.X
OP = mybir.AluOpType
AF = mybir.ActivationFunctionType

C_ROUND = 12582912.0  # 1.5 * 2**23 : fp32 RNE rounding constant
N_CORES = 8


def build_bitnet(D, I, T, n_cores=N_CORES):
    """Per-core SPMD Bass program.

    Per-core I/O: xT [D,T] f32 (token shard, transposed), w1T [D,I] f32
    (full transposed w1), w1s [D/n,I] / w2s [I/n,D] f32 (this core's row
    shards) -> outT [D,T] f32.
    """
    KD = D // 128   # 16  d tiles (layer-1 contraction)
    KI = I // 128   # 64  inner tiles
    TH = T // 2     # 512 matmul moving free dim
    TJ = T // 128   # 8   token tiles for stats transposes
    R1 = D // n_cores   # 256  w1 shard rows (scale partial only)
    R2 = I // n_cores   # 1024 w2 shard rows (quantized here, gathered)
    A1 = R1 // 128  # 2
    A2 = R2 // 128  # 8
    NB1 = KI // 4   # 16  layer-1 weight bands (4 i-strips = 512 cols each)
    inv_cnt = 1.0 / float(D * I)
    sqrt_d = float(np.sqrt(np.float64(D)))
    sqrt_i = float(np.sqrt(np.float64(I)))

    nc = bacc.Bacc("TRN2", num_devices=n_cores)

    xT = nc.dram_tensor("xT", [D, T], F32, kind="ExternalInput")
    w1T = nc.dram_tensor("w1T", [D, I], F32, kind="ExternalInput")
    w1s = nc.dram_tensor("w1s", [R1, I], F32, kind="ExternalInput")
    w2s = nc.dram_tensor("w2s", [R2, D], F32, kind="ExternalInput")
    outT = nc.dram_tensor("outT", [D, T], F32, kind="ExternalOutput")

    h_dram = nc.dram_tensor("h_scratch", [I, T], F32, kind="Internal")
    w2ag_in = nc.dram_tensor("w2ag_in", [R2, D], BF16, kind="Internal")
    w2q_dram = nc.dram_tensor("w2q_ag", [I, D], BF16, kind="Internal",
                              addr_space="Shared")
    ar_in = nc.dram_tensor("ar_in", [8], F32, kind="Internal")
    ar_out = nc.dram_tensor("ar_out", [8], F32, kind="Internal",
                            addr_space="Shared")
    stat_dram = nc.dram_tensor("stat_dram", [6, T], F32, kind="Internal")
    srow_v = stat_dram.ap()                                       # [6, T]
    stok_v = stat_dram.ap().rearrange("r (j p) -> r p j", p=128)  # [6,128,TJ]

    xT_t = xT.ap().rearrange("(k p) t -> k p t", p=128)           # [KD,128,T]
    # layer-1 weight bands: band b = w1T cols [512b, 512b+512), all k-tiles
    w1v = w1T.ap().rearrange("(k p) (b c) -> b p k c", p=128, c=512)
    w1s_ap = w1s.ap()
    w2s_ap = w2s.ap()
    # gathered quant w2 [I, D]: group g = cols [512g, 512g+512)
    w2q_v = w2q_dram.ap().rearrange("(k p) (g c) -> g p k c", p=128, c=512)
    h_w = h_dram.ap().rearrange("(k p) t -> k p t", p=128)
    out_w = outT.ap().rearrange("(k p) t -> k p t", p=128)

    with tile.TileContext(nc) as tc:
        with (
            tc.tile_pool(name="glob", bufs=1) as glob,
            tc.tile_pool(name="psum", bufs=8, space="PSUM") as psum,
            tc.tile_pool(name="stats", bufs=1) as stats,
        ):
            # --- persistent constants & small tiles ---
            ident = glob.tile([128, 128], F32)
            make_identity(nc, ident)
            wsc = glob.tile([128, 4], F32)   # cols: s1, s2, mclip1, mclip2
            qs1_b = glob.tile([128, T], F32, tag="qsb")
            al1_b = glob.tile([128, T], F32, tag="alb")

            def part_reduce(acc, res, op):
                # reduce [128, T] over partitions -> res [128, TJ] tok-part
                for j in range(TJ):
                    trp = psum.tile([128, 128], F32, tag="b", name="trp")
                    nc.tensor.transpose(trp[:, :], acc[:, ts(j, 128)],
                                        ident[:, :])
                    nc.vector.tensor_reduce(
                        out=res[:, j:j + 1], in_=trp[:, :], axis=AX, op=op)

            def finalize_stats(Mx, ssq, mclip_col, sqrt_dim, qs_b, al_b, r0):
                """Mx/ssq [128,TJ] tok-part absmax / sumsq.
                Builds qs_b = 127/max|x| and al_b = per-token dequant scale,
                both broadcast to [128, T]. r0: base row in stat_dram."""
                nrm = stats.tile([128, TJ], F32, name="nrm")
                nc.vector.tensor_scalar(nrm, ssq, 1e-38, None, OP.max)
                nc.scalar.activation(nrm, nrm, AF.Sqrt)
                nc.vector.tensor_scalar(nrm, nrm, 1e-12, None, OP.max)
                inv_n = stats.tile([128, TJ], F32, name="inv_n")
                nc.vector.reciprocal(inv_n, nrm)
                al = stats.tile([128, TJ], F32, name="al")
                nc.vector.tensor_tensor(al, Mx, inv_n, OP.mult)
                nc.vector.tensor_scalar(al, al, sqrt_dim, 1e-5, OP.mult, OP.max)
                nc.vector.tensor_scalar(al, al, wsc[:, mclip_col:mclip_col + 1],
                                        1.0 / 127.0, OP.mult, OP.mult)
                qs = stats.tile([128, TJ], F32, name="qs")
                nc.vector.tensor_scalar(qs, Mx, 1e-30, None, OP.max)
                nc.vector.reciprocal(qs, qs)
                nc.vector.tensor_scalar(qs, qs, 127.0, None, OP.mult)
                nc.sync.dma_start(out=stok_v[r0 + 1], in_=qs[:, :])
                nc.sync.dma_start(out=stok_v[r0 + 2], in_=al[:, :])
                qrow = stats.tile([1, T], F32, name="qrow")
                arow = stats.tile([1, T], F32, name="arow")
                nc.sync.dma_start(out=qrow[:, :], in_=srow_v[r0 + 1:r0 + 2, :])
                nc.sync.dma_start(out=arow[:, :], in_=srow_v[r0 + 2:r0 + 3, :])
                nc.gpsimd.partition_broadcast(qs_b[:, :], qrow[:, :])
                nc.gpsimd.partition_broadcast(al_b[:, :], arow[:, :])

            with tc.tile_pool(name="xqp", bufs=1) as xqp:
                xqT = xqp.tile([128, KD, T], BF16, name="xqT")
                with tc.tile_pool(name="xfp", bufs=1) as xfp:
                    xf = xfp.tile([128, KD, T], F32, name="xf")
                    with tc.tile_pool(name="early", bufs=2) as early:
                        # === Stage A: |w| partials -> tiny AllReduce ===
                        wps = stats.tile([128, 4 * A1 + A2], F32)
                        for a in range(A1):
                            for cq in range(4):
                                wtmp = early.tile([128, I // 4], F32,
                                                  tag="wred", name="wtmp")
                                nc.sync.dma_start(
                                    out=wtmp[:, :],
                                    in_=w1s_ap[128 * a:128 * (a + 1),
                                               ts(cq, I // 4)])
                                nc.vector.tensor_reduce(
                                    out=wps[:, 4 * a + cq:4 * a + cq + 1],
                                    in_=wtmp[:, :], axis=AX,
                                    op=OP.add, apply_absolute_value=True)
                        for a in range(A2):
                            wtmp2 = early.tile([128, D], F32, tag="wred2",
                                               name="wtmp2")
                            nc.sync.dma_start(
                                out=wtmp2[:, :],
                                in_=w2s_ap[128 * a:128 * (a + 1), :])
                            nc.vector.tensor_reduce(
                                out=wps[:, 4 * A1 + a:4 * A1 + a + 1],
                                in_=wtmp2[:, :], axis=AX,
                                op=OP.add, apply_absolute_value=True)
                        wpad = stats.tile([128, 128], F32)
                        nc.vector.memset(wpad, 0.0)
                        nc.vector.reduce_sum(wpad[:, 0:1], wps[:, 0:4 * A1],
                                             axis=AX)
                        nc.vector.reduce_sum(wpad[:, 1:2],
                                             wps[:, 4 * A1:4 * A1 + A2],
                                             axis=AX)
                        trw = psum.tile([128, 128], F32, tag="b", name="trw")
                        nc.tensor.transpose(trw[:, :], wpad[:, :], ident[:, :])
                        wred = stats.tile([8, 1], F32)
                        nc.vector.memset(wred, 0.0)
                        nc.vector.reduce_sum(wred[0:2, :], trw[0:2, :],
                                             axis=AX)
                        nc.sync.dma_start(out=ar_in.ap()[0:8], in_=wred[:, :])
                        nc.gpsimd.collective_compute(
                            "AllReduce", OP.add,
                            replica_groups=[list(range(n_cores))],
                            ins=[ar_in.ap().opt()], outs=[ar_out.ap().opt()])
                        wrow = stats.tile([1, 2], F32)
                        nc.sync.dma_start(out=wrow[:, :], in_=ar_out.ap()[0:2])
                        mrow = stats.tile([1, 4], F32)
                        nc.vector.tensor_scalar(mrow[:, 2:4], wrow[:, :],
                                                inv_cnt, 1e-5,
                                                OP.mult, OP.max)
                        nc.vector.reciprocal(mrow[:, 0:2], mrow[:, 2:4])
                        nc.gpsimd.partition_broadcast(wsc[:, :], mrow[:, :])

                        # === Stage B: x stats + quant (x SBUF-resident) ===
                        for k in range(KD):
                            nc.sync.dma_start(out=xf[:, k, :], in_=xT_t[k])
                        am1p = stats.tile([128, T], F32, tag="amp",
                                          name="am1p")
                        am1n = stats.tile([128, T], F32, tag="amn",
                                          name="am1n")
                        sq1 = stats.tile([128, T], F32, tag="sq", name="sq1")
                        for k in range(KD):
                            xk = xf[:, k, :]
                            if k == 0:
                                nc.vector.tensor_copy(am1p, xk)
                                nc.vector.tensor_copy(am1n, xk)
                            else:
                                nc.vector.tensor_tensor(am1p, xk, am1p,
                                                        OP.max)
                                nc.vector.tensor_tensor(am1n, xk, am1n,
                                                        OP.min)
                            xsq = early.tile([128, T], BF16, tag="xsq",
                                             name="xsq")
                            nc.scalar.activation(xsq, xk, AF.Square)
                            if k == 0:
                                nc.vector.tensor_copy(sq1, xsq)
                            else:
                                nc.vector.tensor_tensor(sq1, xsq, sq1, OP.add)
                        nc.vector.scalar_tensor_tensor(
                            am1n, am1n, -1.0, am1p, OP.mult, OP.max)
                        Mx1 = stats.tile([128, TJ], F32)
                        part_reduce(am1n, Mx1, OP.max)
                        Sq1 = stats.tile([128, TJ], F32)
                        part_reduce(sq1, Sq1, OP.add)
                        finalize_stats(Mx1, Sq1, 2, sqrt_d, qs1_b, al1_b, 0)

                        for k in range(KD):
                            xk2 = early.tile([128, T], F32, tag="xk",
                                             name="xk2")
                            nc.vector.tensor_tensor(xk2, xf[:, k, :], qs1_b,
                                                    OP.mult)
                            nc.vector.tensor_scalar(xqT[:, k, :], xk2,
                                                    C_ROUND, C_ROUND,
                                                    OP.add, OP.subtract)
                    # early + xfp closed: their SBUF is released for L1

                # stats accumulators for h (reuse amp/amn/sq buffers)
                am2p = stats.tile([128, T], F32, tag="amp", name="am2p")
                am2n = stats.tile([128, T], F32, tag="amn", name="am2n")
                sq2 = stats.tile([128, T], F32, tag="sq", name="sq2")

                # ====== Layer 1 (+ w2 shard quant/AllGather overlap) ======
                run_l1(nc, tc, psum, wsc, al1_b, xqT, w1v, w2s_ap,
                       w2ag_in, w2q_dram, h_w, am2p, am2n, sq2,
                       KD, TH, T, NB1, A2, D, n_cores)

            # ---- mid stats finalize ----
            qs2_b = glob.tile([128, T], F32, tag="qsb", name="qs2_b")
            al2_b = glob.tile([128, T], F32, tag="alb", name="al2_b")
            nc.vector.scalar_tensor_tensor(
                am2n, am2n, -1.0, am2p, OP.mult, OP.max)
            Mx2 = stats.tile([128, TJ], F32, name="Mx2")
            part_reduce(am2n, Mx2, OP.max)
            Sq2 = stats.tile([128, TJ], F32, name="Sq2")
            part_reduce(sq2, Sq2, OP.add)
            finalize_stats(Mx2, Sq2, 3, sqrt_i, qs2_b, al2_b, 3)

            # ================= Layer 2 =================
            with tc.tile_pool(name="l2", bufs=2) as l2:
                hq = l2.tile([128, KI, T], BF16, tag="hq", bufs=1, name="hq")
                for g in range(4):
                    p2 = [psum.tile([128, TH], F32, tag="b", name=f"l2p{j}")
                          for j in range(8)]
                    for kc in range(8):
                        w2c = l2.tile([128, 8, 512], BF16, tag="w2c", bufs=3,
                                      name="w2c")
                        nc.sync.dma_start(out=w2c[:, :, :],
                                          in_=w2q_v[g][:, ts(kc, 8), :])
                        for kk in range(8):
                            k = kc * 8 + kk
                            if g == 0:
                                hk = l2.tile([128, T], F32, tag="hrd", bufs=3,
                                             name="hk")
                                nc.sync.dma_start(out=hk[:, :], in_=h_w[k])
                                nc.vector.tensor_tensor(hk, hk, qs2_b,
                                                        OP.mult)
                                nc.vector.tensor_scalar(hq[:, k, :], hk,
                                                        C_ROUND, C_ROUND,
                                                        OP.add, OP.subtract)
                            first = (k == 0)
                            last = (k == KI - 1)
                            for ot in range(4):
                                wap = w2c[:, kk, ts(ot, 128)]
                                nc.tensor.matmul(p2[2 * ot][:, :], wap,
                                                 hq[:, k, 0:TH],
                                                 start=first, stop=last)
                                nc.tensor.matmul(p2[2 * ot + 1][:, :], wap,
                                                 hq[:, k, TH:T],
                                                 start=first, stop=last)
                    for ot in range(4):
                        for hf in range(2):
                            ob = l2.tile([128, TH], F32, tag="ob", bufs=2,
                                         name="ob")
                            nc.vector.tensor_tensor(
                                ob, p2[2 * ot + hf],
                                al2_b[:, ts(hf, TH)], OP.mult)
                            nc.sync.dma_start(
                                out=out_w[4 * g + ot][:, ts(hf, TH)],
                                in_=ob[:, :])

    nc.compile()
    return nc


def run_l1(nc, tc, psum, wsc, al1_b, xqT, w1v, w2s_ap, w2ag_in, w2q_dram,
           h_w, am2p, am2n, sq2, KD, TH, T, NB1, A2, D, n_cores):
    """Layer 1: JIT-quantized w1 bands, k-inner matmul groups, h stats.
    Also issues the w2 shard quant + AllGather early so it overlaps."""

    def quant_w2_shard(pool):
        for a in range(A2):
            wq2f = pool.tile([128, D], F32, tag="w2qf", bufs=1, name="wq2f")
            nc.sync.dma_start(out=wq2f[:, :],
                              in_=w2s_ap[128 * a:128 * (a + 1), :])
            nc.scalar.activation(wq2f, wq2f, AF.Copy,
                                 scale=wsc[:, 1:2], bias=C_ROUND)
            nc.scalar.activation(wq2f, wq2f, AF.Copy, bias=-C_ROUND)
            wq2b = pool.tile([128, D], BF16, tag="w2qb", bufs=1, name="wq2b")
            nc.vector.tensor_scalar(wq2b, wq2f, 1.0, -1.0, OP.min, OP.max)
            nc.sync.dma_start(out=w2ag_in.ap()[128 * a:128 * (a + 1), :],
                              in_=wq2b[:, :])
        nc.gpsimd.collective_compute(
            "AllGather", OP.bypass,
            replica_groups=[list(range(n_cores))],
            ins=[w2ag_in.ap().opt()], outs=[w2q_dram.ap().opt()])

    with tc.tile_pool(name="l1w", bufs=2) as l1w:
        for b in range(NB1):
            # --- JIT quant of band b: w1 cols [512b, 512b+512), all k ---
            w1f = l1w.tile([128, KD, 512], F32, tag="w1f", name="w1f")
            nc.sync.dma_start(out=w1f[:, :, :], in_=w1v[b])
            w1q = l1w.tile([128, KD, 512], BF16, tag="w1q", name="w1q")
            flt = w1f.rearrange("p k c -> p (k c)")
            flq = w1q.rearrange("p k c -> p (k c)")
            nch = 4 if b == 0 else 2  # finer chunks early: start MMs sooner
            cw = (KD * 512) // nch
            for ch in range(nch):
                sl = ts(ch, cw)
                nc.scalar.activation(flt[:, sl], flt[:, sl], AF.Copy,
                                     scale=wsc[:, 0:1], bias=C_ROUND)
                nc.scalar.activation(flt[:, sl], flt[:, sl], AF.Copy,
                                     bias=-C_ROUND)
                nc.vector.tensor_scalar(flq[:, sl], flt[:, sl], 1.0, -1.0,
                                        OP.min, OP.max)
            for sg in range(2):
                pa = [psum.tile([128, TH], F32, tag="b", name=f"l1p{j}")
                      for j in range(4)]
                for k in range(KD):
                    first = (k == 0)
                    last = (k == KD - 1)
                    for ot in range(2):
                        wap = w1q[:, k, ts(sg * 2 + ot, 128)]
                        nc.tensor.matmul(pa[2 * ot][:, :], wap,
                                         xqT[:, k, 0:TH],
                                         start=first, stop=last)
                        nc.tensor.matmul(pa[2 * ot + 1][:, :], wap,
                                         xqT[:, k, TH:T],
                                         start=first, stop=last)
                for ot in range(2):
                    strip = b * 4 + sg * 2 + ot
                    h_sb = l1w.tile([128, T], F32, tag="h", bufs=3,
                                    name="h_sb")
                    nc.vector.tensor_tensor(h_sb[:, 0:TH], pa[2 * ot],
                                            al1_b[:, 0:TH], OP.mult)
                    nc.vector.tensor_tensor(h_sb[:, TH:T], pa[2 * ot + 1],
                                            al1_b[:, TH:T], OP.mult)
                    nc.scalar.activation(h_sb, h_sb, AF.Gelu)
                    nc.sync.dma_start(out=h_w[strip], in_=h_sb[:, :])
                    if strip == 0:
                        nc.vector.tensor_copy(am2p, h_sb)
                        nc.vector.tensor_copy(am2n, h_sb)
                    else:
                        nc.vector.tensor_tensor(am2p, h_sb, am2p, OP.max)
                        nc.vector.tensor_tensor(am2n, h_sb, am2n, OP.min)
                    hsq = l1w.tile([128, T], BF16, tag="hsq", name="hsq")
                    nc.scalar.activation(hsq, h_sb, AF.Square)
                    if strip == 0:
                        nc.vector.tensor_copy(sq2, hsq)
                    else:
                        nc.vector.tensor_tensor(sq2, hsq, sq2, OP.add)
            if b == 2:
                # w2 quant + AllGather: issued here so its scalar/DVE work
                # lands after the first bands' (PE-critical) quant chains,
                # but early enough to finish well before layer 2.
                quant_w2_shard(l1w)


_NC_CACHE = {}


def _get_nc(D, I, T, n_cores):
    key = (D, I, T, n_cores)
    if key not in _NC_CACHE:
        _NC_CACHE[key] = build_bitnet(D, I, T, n_cores)
    return _NC_CACHE[key]


def make_in_maps(x, w1, w2, n_cores=N_CORES):
    """Host-side sharding/layout only (transpose + slicing, no arithmetic)."""
    xf = np.ascontiguousarray(np.asarray(x, dtype=np.float32)).reshape(
        -1, x.shape[-1])
    D = xf.shape[1]
    I = w1.shape[0]
    T = xf.shape[0] // n_cores
    w1T = np.ascontiguousarray(np.asarray(w1, dtype=np.float32).T)  # [D, I]
    w2T = np.ascontiguousarray(np.asarray(w2, dtype=np.float32).T)  # [I, D]
    in_maps = []
    for c in range(n_cores):
        xTc = np.ascontiguousarray(xf[c * T:(c + 1) * T].T)  # [D, T]
        in_maps.append({
            "xT": xTc,
            "w1T": w1T,
            "w1s": np.ascontiguousarray(
                w1T[c * (D // n_cores):(c + 1) * (D // n_cores)]),
            "w2s": np.ascontiguousarray(
                w2T[c * (I // n_cores):(c + 1) * (I // n_cores)]),
        })
    return in_maps, (D, I, T)


def run_spmd(x, w1, w2, trace=False, **kwargs):
    from concourse.bass_utils import run_bass_kernel_spmd

    B, S, D = x.shape
    in_maps, (D, I, T) = make_in_maps(x, w1, w2, N_CORES)
    nc = _get_nc(D, I, T, N_CORES)
    res = run_bass_kernel_spmd(nc, in_maps, core_ids=list(range(N_CORES)),
                               trace=trace, **kwargs)
    outs = [res.results[c]["outT"].T for c in range(N_CORES)]  # each [T, D]
    out = np.concatenate(outs, axis=0).reshape(B, S, D)
    return np.ascontiguousarray(out, dtype=np.float32), res


def kernel(x, w1, w2):
    out, _ = run_spmd(x, w1, w2, trace=False)
    return out


# revision 6
# speedup vs baseline: 1.1918x; 1.0817x over previous
"""BitNet FFN (bitlinear158 -> gelu -> bitlinear158) Trainium2 kernel, v3.

Sharding: data-parallel over tokens across 8 cores (1024 tokens/core).
Layout: tokens on the free axis everywhere; weights stationary in the PE.

v3 structure: NO weight AllGathers at all -- each core JIT-quantizes both
full weight matrices locally (w1 in 16 bands during layer 1, w2 in 128
2-strip chunks during layer 2; the scalar engine is otherwise idle in each
phase).  The only cross-core traffic is two 8-float AllReduces for the
weight-quant scales (mean|w|): the w1 one is prioritized so the PE starts
~100us in, the w2 one fires right after and has ~600us of slack.  Matmuls
are issued k-innermost in PSUM ping-pong groups, consecutive matmuls share
the stationary operand (2 MMs per LDWEIGHTS).

Math notes (exactness):
  - activation quant ints = round(x * 127 / max|x|)  (the rms-norm cancels)
  - weight quant ternary = clip(round(w / clip(mean|w|,1e-5)), -1, 1)
  - both exactly representable in bf16; PSUM accumulates integer products
    exactly in fp32, so the matmuls are exact.
  - per-token output scale alpha = clip(max|x|*sqrt(d)/||x||, 1e-5)
      * clip(mean|w|,1e-5) / 127 applied on PSUM before gelu.
  - round-to-nearest-even via fp32 (t + 1.5*2^23) - 1.5*2^23, matching
    jnp.round; round-then-clip == reference clip(round(t), -1, 1).
  - mean|w| needs the full tensor: each core reduces its row-shard, then a
    tiny AllReduce combines the partial sums.
"""

import sys

for _p in ("/opt/trn_rl_repo", "/opt/trn_rl_repo/concourse"):
    if _p not in sys.path:
        sys.path.insert(0, _p)

import numpy as np

import concourse.bass as bass
import concourse.bacc as bacc
import concourse.mybir as mybir
import concourse.tile as tile
from concourse.bass import ts
from concourse.masks import make_identity

F32 = mybir.dt.float32
BF16 = mybir.dt.bfloat16
AX = mybir.AxisListType.X
OP = mybir.AluOpType
AF = mybir.ActivationFunctionType

C_ROUND = 12582912.0  # 1.5 * 2**23 : fp32 RNE rounding constant
N_CORES = 8


def build_bitnet(D, I, T, n_cores=N_CORES):
    """Per-core SPMD Bass program.

    Per-core I/O: xT [D,T] f32 (token shard, transposed), w1T [D,I] /
    w2T [I,D] f32 (full transposed weights), w1s [D/n,I] / w2s [I/n,D] f32
    (this core's row shards, for the mean|w| partials) -> outT [D,T] f32.
    """
    KD = D // 128   # 16  d tiles (layer-1 contraction)
    KI = I // 128   # 64  inner tiles
    TH = T // 2     # 512 matmul moving free dim
    TJ = T // 128   # 8   token tiles for stats transposes
    R1 = D // n_cores   # 256  w1 shard rows
    R2 = I // n_cores   # 1024 w2 shard rows
    A1 = R1 // 128  # 2
    A2 = R2 // 128  # 8
    NB1 = KI // 4   # 16  layer-1 weight bands (4 i-strips = 512 cols each)
    inv_cnt = 1.0 / float(D * I)
    sqrt_d = float(np.sqrt(np.float64(D)))
    sqrt_i = float(np.sqrt(np.float64(I)))

    nc = bacc.Bacc("TRN2", num_devices=n_cores)

    xT = nc.dram_tensor("xT", [D, T], F32, kind="ExternalInput")
    w1T = nc.dram_tensor("w1T", [D, I], F32, kind="ExternalInput")
    w2T = nc.dram_tensor("w2T", [I, D], F32, kind="ExternalInput")
    w1s = nc.dram_tensor("w1s", [R1, I], F32, kind="ExternalInput")
    w2s = nc.dram_tensor("w2s", [R2, D], F32, kind="ExternalInput")
    outT = nc.dram_tensor("outT", [D, T], F32, kind="ExternalOutput")

    h_dram = nc.dram_tensor("h_scratch", [I, T], F32, kind="Internal")
    ar1_in = nc.dram_tensor("ar1_in", [8], F32, kind="Internal")
    ar1_out = nc.dram_tensor("ar1_out", [8], F32, kind="Internal",
                             addr_space="Shared")
    ar2_in = nc.dram_tensor("ar2_in", [8], F32, kind="Internal")
    ar2_out = nc.dram_tensor("ar2_out", [8], F32, kind="Internal",
                             addr_space="Shared")
    stat_dram = nc.dram_tensor("stat_dram", [6, T], F32, kind="Internal")
    srow_v = stat_dram.ap()                                       # [6, T]
    stok_v = stat_dram.ap().rearrange("r (j p) -> r p j", p=128)  # [6,128,TJ]

    xT_t = xT.ap().rearrange("(k p) t -> k p t", p=128)           # [KD,128,T]
    # layer-1 weight bands: band b = w1T cols [512b, 512b+512), all k-tiles
    w1v = w1T.ap().rearrange("(k p) (b c) -> b p k c", p=128, c=512)
    # layer-2 weight groups: group g = w2T cols [512g, 512g+512)
    w2v = w2T.ap().rearrange("(k p) (g c) -> g p k c", p=128, c=512)
    w1s_ap = w1s.ap()
    w2s_ap = w2s.ap()
    h_w = h_dram.ap().rearrange("(k p) t -> k p t", p=128)
    h_r2 = h_dram.ap().rearrange("(k q p) t -> k p q t", q=2, p=128)
    out_w = outT.ap().rearrange("(k p) t -> k p t", p=128)

    with tile.TileContext(nc) as tc:
        with (
            tc.tile_pool(name="glob", bufs=1) as glob,
            tc.tile_pool(name="psum", bufs=8, space="PSUM") as psum,
            tc.tile_pool(name="stats", bufs=1) as stats,
        ):
            # --- persistent constants & small tiles ---
            ident = glob.tile([128, 128], F32)
            make_identity(nc, ident)
            wsc1 = glob.tile([128, 2], F32, name="wsc1")  # cols: s1, mclip1
            wsc2 = glob.tile([128, 2], F32, name="wsc2")  # cols: s2, mclip2
            qs1_b = glob.tile([128, T], F32, tag="qsb")
            al1_b = glob.tile([128, T], F32, tag="alb")

            def part_reduce(acc, res, op):
                # reduce [128, T] over partitions -> res [128, TJ] tok-part
                for j in range(TJ):
                    trp = psum.tile([128, 128], F32, tag="b", name="trp")
                    nc.tensor.transpose(trp[:, :], acc[:, ts(j, 128)],
                                        ident[:, :])
                    nc.vector.tensor_reduce(
                        out=res[:, j:j + 1], in_=trp[:, :], axis=AX, op=op)

            def finalize_stats(Mx, ssq, mclip, sqrt_dim, qs_b, al_b, r0):
                """Mx/ssq [128,TJ] tok-part absmax / sumsq.
                Builds qs_b = 127/max|x| and al_b = per-token dequant scale,
                both broadcast to [128, T]. r0: base row in stat_dram."""
                nrm = stats.tile([128, TJ], F32, name="nrm")
                nc.vector.tensor_scalar(nrm, ssq, 1e-38, None, OP.max)
                nc.scalar.activation(nrm, nrm, AF.Sqrt)
                nc.vector.tensor_scalar(nrm, nrm, 1e-12, None, OP.max)
                inv_n = stats.tile([128, TJ], F32, name="inv_n")
                nc.vector.reciprocal(inv_n, nrm)
                al = stats.tile([128, TJ], F32, name="al")
                nc.vector.tensor_tensor(al, Mx, inv_n, OP.mult)
                nc.vector.tensor_scalar(al, al, sqrt_dim, 1e-5, OP.mult, OP.max)
                nc.vector.tensor_scalar(al, al, mclip,
                                        1.0 / 127.0, OP.mult, OP.mult)
                qs = stats.tile([128, TJ], F32, name="qs")
                nc.vector.tensor_scalar(qs, Mx, 1e-30, None, OP.max)
                nc.vector.reciprocal(qs, qs)
                nc.vector.tensor_scalar(qs, qs, 127.0, None, OP.mult)
                nc.sync.dma_start(out=stok_v[r0 + 1], in_=qs[:, :])
                nc.sync.dma_start(out=stok_v[r0 + 2], in_=al[:, :])
                qrow = stats.tile([1, T], F32, name="qrow")
                arow = stats.tile([1, T], F32, name="arow")
                nc.sync.dma_start(out=qrow[:, :], in_=srow_v[r0 + 1:r0 + 2, :])
                nc.sync.dma_start(out=arow[:, :], in_=srow_v[r0 + 2:r0 + 3, :])
                nc.gpsimd.partition_broadcast(qs_b[:, :], qrow[:, :])
                nc.gpsimd.partition_broadcast(al_b[:, :], arow[:, :])

            def all_reduce_scale(wred8, ar_in, ar_out, wsc):
                # wred8 [8,1]: row0 = this core's partial |w| sum
                nc.sync.dma_start(out=ar_in.ap()[0:8], in_=wred8[:, :])
                nc.gpsimd.collective_compute(
                    "AllReduce", OP.add,
                    replica_groups=[list(range(n_cores))],
                    ins=[ar_in.ap().opt()], outs=[ar_out.ap().opt()])
                wrow = stats.tile([1, 1], F32, name="wrow")
                nc.sync.dma_start(out=wrow[:, :], in_=ar_out.ap()[0:1])
                mrow = stats.tile([1, 2], F32, name="mrow")
                nc.vector.tensor_scalar(mrow[:, 1:2], wrow[:, :], inv_cnt,
                                        1e-5, OP.mult, OP.max)
                nc.vector.reciprocal(mrow[:, 0:1], mrow[:, 1:2])
                nc.gpsimd.partition_broadcast(wsc[:, :], mrow[:, :])

            with tc.tile_pool(name="xqp", bufs=1) as xqp:
                xqT = xqp.tile([128, KD, T], BF16, name="xqT")
                with tc.tile_pool(name="xfp", bufs=1) as xfp:
                    xf = xfp.tile([128, KD, T], F32, name="xf")
                    with tc.tile_pool(name="early", bufs=2) as early:
                        # --- DMA priority: w1s first (gates AR1), then x,
                        # then w2s (gates AR2, which has slack).  Each chunk's
                        # reduce follows its dma (bufs rotation safety). ---
                        wps = stats.tile([128, 16], F32)
                        for a in range(A1):
                            for cq in range(4):
                                i = 4 * a + cq
                                wt = early.tile([128, I // 4], F32, bufs=3,
                                                tag="wred", name="wt")
                                nc.sync.dma_start(
                                    out=wt[:, :],
                                    in_=w1s_ap[128 * a:128 * (a + 1),
                                               ts(cq, I // 4)])
                                nc.vector.tensor_reduce(
                                    out=wps[:, i:i + 1], in_=wt[:, :],
                                    axis=AX, op=OP.add,
                                    apply_absolute_value=True)
                        for k in range(KD):
                            nc.sync.dma_start(out=xf[:, k, :], in_=xT_t[k])

                        # === AR1 chain: w1 |w| partial sums ===
                        wpad = stats.tile([128, 128], F32)
                        nc.vector.memset(wpad, 0.0)
                        nc.vector.reduce_sum(wpad[:, 0:1], wps[:, 0:8],
                                             axis=AX)
                        trw = psum.tile([128, 128], F32, tag="b", name="trw")
                        nc.tensor.transpose(trw[:, :], wpad[:, :], ident[:, :])
                        wred8 = stats.tile([8, 1], F32, name="wred8")
                        nc.vector.memset(wred8, 0.0)
                        nc.vector.reduce_sum(wred8[0:1, :], trw[0:1, :],
                                             axis=AX)
                        all_reduce_scale(wred8, ar1_in, ar1_out, wsc1)

                        # === Stage B: x stats ===
                        am1p = stats.tile([128, T], F32, tag="amp",
                                          name="am1p")
                        am1n = stats.tile([128, T], F32, tag="amn",
                                          name="am1n")
                        sq1 = stats.tile([128, T], F32, tag="sq", name="sq1")
                        for k in range(KD):
                            xk = xf[:, k, :]
                            if k == 0:
                                nc.vector.tensor_copy(am1p, xk)
                                nc.vector.tensor_copy(am1n, xk)
                            else:
                                nc.vector.tensor_tensor(am1p, xk, am1p,
                                                        OP.max)
                                nc.vector.tensor_tensor(am1n, xk, am1n,
                                                        OP.min)
                            xsq = early.tile([128, T], BF16, tag="xsq",
                                             name="xsq")
                            nc.scalar.activation(xsq, xk, AF.Square)
                            if k == 0:
                                nc.vector.tensor_copy(sq1, xsq)
                            else:
                                nc.vector.tensor_tensor(sq1, xsq, sq1, OP.add)
                        nc.vector.scalar_tensor_tensor(
                            am1n, am1n, -1.0, am1p, OP.mult, OP.max)
                        Mx1 = stats.tile([128, TJ], F32)
                        part_reduce(am1n, Mx1, OP.max)
                        Sq1 = stats.tile([128, TJ], F32)
                        part_reduce(sq1, Sq1, OP.add)
                        finalize_stats(Mx1, Sq1, wsc1[:, 1:2], sqrt_d,
                                       qs1_b, al1_b, 0)

                        # === AR2 chain: w2 |w| partials (after the qs1/al1
                        # broadcasts in the gpsimd queue; ~600us of slack) ===
                        wps2 = stats.tile([128, 8], F32, name="wps2")
                        for a in range(A2):
                            wt2 = early.tile([128, D], F32, tag="wred2",
                                             name="wt2")
                            nc.sync.dma_start(
                                out=wt2[:, :],
                                in_=w2s_ap[128 * a:128 * (a + 1), :])
                            nc.vector.tensor_reduce(
                                out=wps2[:, a:a + 1], in_=wt2[:, :], axis=AX,
                                op=OP.add, apply_absolute_value=True)
                        wpad2 = stats.tile([128, 128], F32, name="wpad2")
                        nc.vector.memset(wpad2, 0.0)
                        nc.vector.reduce_sum(wpad2[:, 0:1], wps2[:, 0:8],
                                             axis=AX)
                        trw2 = psum.tile([128, 128], F32, tag="b", name="trw2")
                        nc.tensor.transpose(trw2[:, :], wpad2[:, :],
                                            ident[:, :])
                        wred8b = stats.tile([8, 1], F32, name="wred8b")
                        nc.vector.memset(wred8b, 0.0)
                        nc.vector.reduce_sum(wred8b[0:1, :], trw2[0:1, :],
                                             axis=AX)
                        all_reduce_scale(wred8b, ar2_in, ar2_out, wsc2)

                        # === x quant ===
                        for k in range(KD):
                            xk2 = early.tile([128, T], F32, tag="xk",
                                             name="xk2")
                            nc.vector.tensor_tensor(xk2, xf[:, k, :], qs1_b,
                                                    OP.mult)
                            nc.vector.tensor_scalar(xqT[:, k, :], xk2,
                                                    C_ROUND, C_ROUND,
                                                    OP.add, OP.subtract)
                    # early + xfp closed: their SBUF is released for L1

                # stats accumulators for h (reuse amp/amn/sq buffers)
                am2p = stats.tile([128, T], F32, tag="amp", name="am2p")
                am2n = stats.tile([128, T], F32, tag="amn", name="am2n")
                sq2 = stats.tile([128, T], F32, tag="sq", name="sq2")

                # ================= Layer 1 =================
                with tc.tile_pool(name="l1w", bufs=2) as l1w:
                    for b in range(NB1):
                        # JIT quant band b: w1 cols [512b, 512b+512), all k
                        w1f = l1w.tile([128, KD, 512], F32, tag="w1f",
                                       name="w1f")
                        nc.sync.dma_start(out=w1f[:, :, :], in_=w1v[b])
                        w1q = l1w.tile([128, KD, 512], BF16, tag="w1q",
                                       name="w1q")
                        flt = w1f.rearrange("p k c -> p (k c)")
                        flq = w1q.rearrange("p k c -> p (k c)")
                        nch = 4 if b == 0 else 2
                        cw = (KD * 512) // nch
                        for ch in range(nch):
                            sl = ts(ch, cw)
                            nc.scalar.activation(flt[:, sl], flt[:, sl],
                                                 AF.Copy, scale=wsc1[:, 0:1],
                                                 bias=C_ROUND)
                            nc.scalar.activation(flt[:, sl], flt[:, sl],
                                                 AF.Copy, bias=-C_ROUND)
                            nc.vector.tensor_scalar(flq[:, sl], flt[:, sl],
                                                    1.0, -1.0, OP.min, OP.max)
                        for sg in range(2):
                            pa = [psum.tile([128, TH], F32, tag="b",
                                            name=f"l1p{j}") for j in range(4)]
                            for k in range(KD):
                                first = (k == 0)
                                last = (k == KD - 1)
                                for ot in range(2):
                                    wap = w1q[:, k, ts(sg * 2 + ot, 128)]
                                    nc.tensor.matmul(pa[2 * ot][:, :], wap,
                                                     xqT[:, k, 0:TH],
                                                     start=first, stop=last)
                                    nc.tensor.matmul(pa[2 * ot + 1][:, :],
                                                     wap, xqT[:, k, TH:T],
                                                     start=first, stop=last)
                            for ot in range(2):
                                strip = b * 4 + sg * 2 + ot
                                h_sb = l1w.tile([128, T], F32, tag="h",
                                                bufs=4, name="h_sb")
                                nc.vector.tensor_tensor(h_sb[:, 0:TH],
                                                        pa[2 * ot],
                                                        al1_b[:, 0:TH],
                                                        OP.mult)
                                nc.vector.tensor_tensor(h_sb[:, TH:T],
                                                        pa[2 * ot + 1],
                                                        al1_b[:, TH:T],
                                                        OP.mult)
                                nc.scalar.activation(h_sb, h_sb, AF.Gelu)
                                nc.sync.dma_start(out=h_w[strip],
                                                  in_=h_sb[:, :])
                                if strip == 0:
                                    nc.vector.tensor_copy(am2p, h_sb)
                                    nc.vector.tensor_copy(am2n, h_sb)
                                else:
                                    nc.vector.tensor_tensor(am2p, h_sb, am2p,
                                                            OP.max)
                                    nc.vector.tensor_tensor(am2n, h_sb, am2n,
                                                            OP.min)
                                hsq = l1w.tile([128, T], BF16, tag="hsq",
                                               name="hsq")
                                nc.scalar.activation(hsq, h_sb, AF.Square)
                                if strip == 0:
                                    nc.vector.tensor_copy(sq2, hsq)
                                else:
                                    nc.vector.tensor_tensor(sq2, hsq, sq2,
                                                            OP.add)

            # ---- mid stats finalize ----
            qs2_b = glob.tile([128, T], F32, tag="qsb", name="qs2_b")
            al2_b = glob.tile([128, T], F32, tag="alb", name="al2_b")
            nc.vector.scalar_tensor_tensor(
                am2n, am2n, -1.0, am2p, OP.mult, OP.max)
            Mx2 = stats.tile([128, TJ], F32, name="Mx2")
            part_reduce(am2n, Mx2, OP.max)
            Sq2 = stats.tile([128, TJ], F32, name="Sq2")
            part_reduce(sq2, Sq2, OP.add)
            finalize_stats(Mx2, Sq2, wsc2[:, 1:2], sqrt_i, qs2_b, al2_b, 3)

            # ================= Layer 2 =================
            with tc.tile_pool(name="l2", bufs=2) as l2:
                hq = l2.tile([128, KI, T], BF16, tag="hq", bufs=1, name="hq")
                for g in range(4):
                    p2 = [psum.tile([128, TH], F32, tag="b", name=f"l2p{j}")
                          for j in range(8)]
                    for kc in range(KI // 2):
                        # JIT quant w2 chunk: strips 2kc..2kc+1 of group g
                        w2f = l2.tile([128, 2, 512], F32, tag="w2f",
                                      name="w2f")
                        nc.sync.dma_start(out=w2f[:, :, :],
                                          in_=w2v[g][:, ts(kc, 2), :])
                        w2q = l2.tile([128, 2, 512], BF16, tag="w2q", bufs=3,
                                      name="w2q")
                        f2t = w2f.rearrange("p k c -> p (k c)")
                        f2q = w2q.rearrange("p k c -> p (k c)")
                        nc.scalar.activation(f2t, f2t, AF.Copy,
                                             scale=wsc2[:, 0:1], bias=C_ROUND)
                        nc.scalar.activation(f2t, f2t, AF.Copy, bias=-C_ROUND)
                        nc.vector.tensor_scalar(f2q, f2t, 1.0, -1.0,
                                                OP.min, OP.max)
                        if g == 0:
                            # JIT quant hq strips 2kc..2kc+1 (fused 2-strip)
                            hk = l2.tile([128, 2, T], F32, tag="hrd", bufs=2,
                                         name="hk")
                            nc.sync.dma_start(out=hk[:, :, :], in_=h_r2[kc])
                            nc.vector.tensor_tensor(hk[:, 0, :], hk[:, 0, :],
                                                    qs2_b, OP.mult)
                            nc.vector.tensor_tensor(hk[:, 1, :], hk[:, 1, :],
                                                    qs2_b, OP.mult)
                            hkf = hk.rearrange("p q t -> p (q t)")
                            hqf = hq[:, ts(kc, 2), :].rearrange(
                                "p q t -> p (q t)")
                            if kc % 8 < 5:
                                # round on scalar (2 ops) to offload DVE
                                nc.scalar.activation(hkf, hkf, AF.Copy,
                                                     bias=C_ROUND)
                                nc.scalar.activation(hqf, hkf, AF.Copy,
                                                     bias=-C_ROUND)
                            else:
                                nc.vector.tensor_scalar(hqf, hkf, C_ROUND,
                                                        C_ROUND, OP.add,
                                                        OP.subtract)
                        for kk in range(2):
                            k = kc * 2 + kk
                            first = (k == 0)
                            last = (k == KI - 1)
                            for ot in range(4):
                                wap = w2q[:, kk, ts(ot, 128)]
                                nc.tensor.matmul(p2[2 * ot][:, :], wap,
                                                 hq[:, k, 0:TH],
                                                 start=first, stop=last)
                                nc.tensor.matmul(p2[2 * ot + 1][:, :], wap,
                                                 hq[:, k, TH:T],
                                                 start=first, stop=last)
                    for ot in range(4):
                        for hf in range(2):
                            ob = l2.tile([128, TH], F32, tag="ob", bufs=2,
                                         name="ob")
                            nc.vector.tensor_tensor(
                                ob, p2[2 * ot + hf],
                                al2_b[:, ts(hf, TH)], OP.mult)
                            nc.sync.dma_start(
                                out=out_w[4 * g + ot][:, ts(hf, TH)],
                                in_=ob[:, :])

    nc.compile()
    return nc


_NC_CACHE = {}


def _get_nc(D, I, T, n_cores):
    key = (D, I, T, n_cores)
    if key not in _NC_CACHE:
        _NC_CACHE[key] = build_bitnet(D, I, T, n_cores)
    return _NC_CACHE[key]


def make_in_maps(x, w1, w2, n_cores=N_CORES):
    """Host-side sharding/layout only (transpose + slicing, no arithmetic)."""
    xf = np.ascontiguousarray(np.asarray(x, dtype=np.float32)).reshape(
        -1, x.shape[-1])
    D = xf.shape[1]
    I = w1.shape[0]
    T = xf.shape[0] // n_cores
    w1T = np.ascontiguousarray(np.asarray(w1, dtype=np.float32).T)  # [D, I]
    w2T = np.ascontiguousarray(np.asarray(w2, dtype=np.float32).T)  # [I, D]
    in_maps = []
    for c in range(n_cores):
        xTc = np.ascontiguousarray(xf[c * T:(c + 1) * T].T)  # [D, T]
        in_maps.append({
            "xT": xTc,
            "w1T": w1T,
            "w2T": w2T,
            "w1s": np.ascontiguousarray(
                w1T[c * (D // n_cores):(c + 1) * (D // n_cores)]),
            "w2s": np.ascontiguousarray(
                w2T[c * (I // n_cores):(c + 1) * (I // n_cores)]),
        })
    return in_maps, (D, I, T)


def run_spmd(x, w1, w2, trace=False, **kwargs):
    from concourse.bass_utils import run_bass_kernel_spmd

    B, S, D = x.shape
    in_maps, (D, I, T) = make_in_maps(x, w1, w2, N_CORES)
    nc = _get_nc(D, I, T, N_CORES)
    res = run_bass_kernel_spmd(nc, in_maps, core_ids=list(range(N_CORES)),
                               trace=trace, **kwargs)
    outs = [res.results[c]["outT"].T for c in range(N_CORES)]  # each [T, D]
    out = np.concatenate(outs, axis=0).reshape(B, S, D)
    return np.ascontiguousarray(out, dtype=np.float32), res


def kernel(x, w1, w2):
    out, _ = run_spmd(x, w1, w2, trace=False)
    return out
